# revision 34
# baseline (speedup 1.0000x reference)
"""AdaIN (segment mean/std + EMA of style stats) on 8 TRN2 NeuronCores — v5.

vs v4 baseline (347us -> ~296us):
  - pass-2 FMA re-associated as out = a (.) (x + b/a): the b'-gather
    matmul PSUM-accumulates an identity-matmul copy of x, so per row the
    elementwise work is one ACT evac (x+b' -> bf16) + one DVE mult
    (a_psum * evac -> bf16 out) instead of evac + mult + add.  For every
    other gather group the +x runs as a GPSIMD add on the evac'd b'
    instead of the identity matmul (spreads work onto the idle engine).
  - pair-granularity apply: two groups share one [128,1024] a-gather PSUM
    tile, one xb pair tile, and a single FD-1024 DVE mult / pair.
  - one-hot strips via per-partition tensor_scalar is_equal straight from
    the broadcast-matmul PSUM (no separate iota broadcast tile).
  - ciT8 regroup DMA layout is k-major so each of the 16 transposed-index
    regroup writes is a contiguous 2KB-per-partition HBM write (the
    q-major layout wrote 256B runs that crawled ~90us and, via a pool
    barrier, stalled the whole kernel).
  - identity/iota consts are built before any bulk-load dma_start is
    emitted (the gpsimd queue paces with the DMA pipeline, so anything
    behind the loads starts ~40us late).
  - a dummy AllReduce triggered before the bulk stream absorbs the
    first-collective warmup; the single combined stats AR at the end of
    pass 1 then completes in ~10-20us, with N_PRE pass-2 preps hoisted
    into its latency window.
  - pass-1 rhs builds alternate engines per chunk (DVE copy + ACT square
    / ACT copy + DVE square) — both run at 1-2x, and alternation keeps
    either engine from pacing the stream; content loads are 64-block
    grained with a fine-grained tail and the pass-1 chunk plan tapers so
    the stats tail drains quickly into the AR.
"""

import os
import sys

import numpy as np

for _p in ("/opt/trn_rl_repo",):
    if _p not in sys.path and os.path.isdir(_p):
        sys.path.insert(0, _p)

from concourse import bacc, bass, bass_utils, masks, mybir, tile

F32 = mybir.dt.float32
BF16 = mybir.dt.bfloat16
I32 = mybir.dt.int32
I16 = mybir.dt.int16

N_CORES = 8
C = 64
B = 16
ALPHA = 0.1
EPS = 1e-8

RC = 128 * 977  # per-core content rows (125056; 8*RC >= 1M)
RS = 128 * 245  # per-core style rows (31360; 8*RS >= 250K)

CH = 32    # pass-1 chunk: blocks per onehot/square tile
MMB = 2    # pass-1 chunks per matmul burst
CH2 = 32   # pass-2 chunk: blocks per out tile (4 gathers of 8)
N_PRE = 9  # pass-2 chunks whose prep is hoisted into the content-AR window


def _ema_lhsT() -> np.ndarray:
    L = np.zeros((B, B), np.float64)
    for b in range(B):
        L[b, 0] = (1.0 - ALPHA) ** b
        for j in range(1, b + 1):
            L[b, j] = ALPHA * (1.0 - ALPHA) ** (b - j)
    return np.ascontiguousarray(L.T).astype(np.float32)


def _e8() -> np.ndarray:
    # e8[g, 16g+s] = 1 : broadcast row g of a K=8 rhs to partitions 16g..16g+16
    e = np.zeros((8, 128), np.float32)
    for g in range(8):
        e[g, 16 * g : 16 * g + 16] = 1.0
    return e


def _s4() -> np.ndarray:
    # s4[32j+s, s] = 1 (s<16): sum the 4 col-group partial stats
    s = np.zeros((128, B), np.float32)
    for j in range(4):
        for t in range(B):
            s[32 * j + t, t] = 1.0
    return s


def _io16() -> np.ndarray:
    return np.broadcast_to(np.arange(B, dtype=np.int16), (128, B)).copy()


def _pid16() -> np.ndarray:
    return (np.arange(128, dtype=np.float32) % 16).reshape(128, 1)


def _chunks(total: int, step: int):
    t0 = 0
    while t0 < total:
        yield t0, min(step, total - t0)
        t0 += step


def _load_plan(ntc: int):
    # 64-block (2MB) loads with a fine-grained tail so the last pass-1
    # chunks aren't gated on one huge DMA completion.
    plan = []
    t = 0
    while ntc - t > 80:
        plan.append((t, 64))
        t += 64
    rem = ntc - t
    for sz in (32, 16, 8, 8, 4, 4, 2, 2, 1, 1):
        if rem <= 0:
            break
        take = min(sz, rem)
        plan.append((t, take))
        t += take
        rem -= take
    while rem > 0:
        plan.append((t, 1))
        t += 1
        rem -= 1
    return plan


def build_nc(rc: int = RC, rs: int = RS, n_cores: int = N_CORES):
    ntc = rc // 128
    nts = rs // 128
    ntc_pad = ((ntc + 7) // 8) * 8
    ntc_t = ((ntc + 127) // 128) * 128

    nc = bacc.Bacc(
        "TRN2", target_bir_lowering=False, debug=False, num_devices=n_cores
    )
    cf = nc.dram_tensor("cf", [rc, C], F32, kind="ExternalInput")
    ci = nc.dram_tensor("ci", [rc], I32, kind="ExternalInput")
    sf = nc.dram_tensor("sf", [rs, C], F32, kind="ExternalInput")
    si = nc.dram_tensor("si", [rs], I32, kind="ExternalInput")
    el = nc.dram_tensor("el", [B, B], F32, kind="ExternalInput")
    e8 = nc.dram_tensor("e8", [8, 128], F32, kind="ExternalInput")
    s4 = nc.dram_tensor("s4", [128, B], F32, kind="ExternalInput")
    io16 = nc.dram_tensor("io16", [128, B], I16, kind="ExternalInput")
    pid16 = nc.dram_tensor("pid16", [128, 1], F32, kind="ExternalInput")
    out = nc.dram_tensor("out", [rc, C], BF16, kind="ExternalOutput")

    cf_v = cf.ap().rearrange("(p n) d -> p n d", p=128)
    ci_v = ci.ap().rearrange("(p n) -> p n", p=128)
    sf_v = sf.ap().rearrange("(p n) d -> p n d", p=128)
    si_v = si.ap().rearrange("(p n) -> p n", p=128)
    out_v = out.ap().rearrange("(p n) d -> p n d", p=128)

    def out_q(i):
        return nc.sync if i % 2 == 0 else nc.scalar

    with tile.TileContext(nc) as tc:
        with (
            tc.tile_pool(name="const", bufs=1) as constp,
            tc.tile_pool(name="cache", bufs=1) as cachep,
            tc.tile_pool(name="dram", bufs=1, space="DRAM") as dramp,
        ):
            # identity built FIRST: its gpsimd ops must not queue behind the
            # bulk-load dma_starts (each paces with the DMA pipeline).
            ident = constp.tile([128, 128], BF16)
            masks.make_identity(nc, ident[:])
            ones_sb = constp.tile([128, 1], BF16)
            nc.gpsimd.memset(ones_sb[:], 1.0)

            # ---------- bulk loads, all on the gpsimd SWDGE queue so the
            # descriptor order controls arrival: indices first, then style,
            # then content.  Feature loads are f32->bf16 cast-DMAs into
            # resident bf16 caches. ----------
            ci_sb = constp.tile([128, ntc], I32)
            nc.gpsimd.dma_start(ci_sb[:], ci_v)
            si_sb = constp.tile([128, nts], I32)
            nc.gpsimd.dma_start(si_sb[:], si_v)
            # dummy AllReduce, triggered before the bulk stream: absorbs the
            # first-collective warmup so the real AR at the end runs ~10us.
            din = dramp.tile([B, B], F32, tag="din")
            dout = dramp.tile([B, B], F32, tag="dout")
            nc.sync.dma_start(din[:], el.ap())
            nc.gpsimd.collective_compute(
                "AllReduce",
                mybir.AluOpType.add,
                replica_groups=[list(range(n_cores))],
                ins=[din.opt()],
                outs=[dout.opt()],
            )
            sxcache = cachep.tile([128, nts, C], BF16)
            for l0, nl in _chunks(nts, 64):
                nc.gpsimd.dma_start(
                    sxcache[:, l0 : l0 + nl, :], sf_v[:, l0 : l0 + nl, :]
                )
            # content loads are split around the style-AR trigger (the
            # collective rides the gpsimd queue): loads emitted before it
            # keep the SDMA engines fed while the trigger waits.
            xcache = cachep.tile([128, ntc_pad, C], BF16)
            load_list = _load_plan(ntc)

            def emit_content_loads(lo, hi):
                for l0, nl in load_list[lo:hi]:
                    nc.gpsimd.dma_start(
                        xcache[:, l0 : l0 + nl, :], cf_v[:, l0 : l0 + nl, :]
                    )

            emit_content_loads(0, None)

            # ---------- constants ----------
            el_sb = constp.tile([B, B], F32)
            nc.sync.dma_start(el_sb[:], el.ap())
            e8_sbf = constp.tile([8, 128], F32)
            nc.sync.dma_start(e8_sbf[:], e8.ap())
            e8_sb = constp.tile([8, 128], BF16)
            nc.vector.tensor_copy(e8_sb[:], e8_sbf[:])
            s4_sb = constp.tile([128, B], F32)
            nc.sync.dma_start(s4_sb[:], s4.ap())
            io16_sb = constp.tile([128, B], I16)
            nc.sync.dma_start(io16_sb[:], io16.ap())
            pid_sb = constp.tile([128, 1], F32)
            nc.sync.dma_start(pid_sb[:], pid16.ap())

            if ntc_pad > ntc:
                nc.vector.memset(xcache[:, ntc:ntc_pad, :], 0.0)

            # coef_bd zero-fill happens early (off the post-AR critical path)
            coef_bd_a = constp.tile([128, 8 * C], BF16)
            coef_bd_b = constp.tile([128, 8 * C], BF16)
            nc.vector.memset(coef_bd_a[:], 0.0)
            nc.vector.memset(coef_bd_b[:], 0.0)

            # ---------- transposed indices -> ciT8 in DRAM, k-major layout:
            # ciT8[g, (k*Q+q)*128 + p] = ciT[8k+g, 128q+p] = idx of lane p in
            # block 128q+8k+g.  k-major makes each regroup DMA a contiguous
            # 2KB-per-partition write (the q-major layout wrote 256B runs,
            # which crawl on HBM). ----------
            NQ = ntc_t // 128
            idxbf = constp.tile([128, ntc_t], BF16)
            if ntc_t > ntc:
                nc.vector.memset(idxbf[:, ntc:ntc_t], float(B))
            nc.vector.tensor_copy(idxbf[:, 0:ntc], ci_sb[:])
            ciT = constp.tile([128, ntc_t], BF16)
            ciT8_d = dramp.tile([8, 16 * NQ * 128], BF16, tag="ciT8")
            with tc.tile_pool(name="ps_tr", bufs=2, space="PSUM") as pstr:
                for q in range(NQ):
                    psT = pstr.tile([128, 128], BF16, tag="tr")
                    nc.tensor.transpose(
                        psT[:], idxbf[:, 128 * q : 128 * (q + 1)], ident[:]
                    )
                    nc.scalar.copy(ciT[:, 128 * q : 128 * (q + 1)], psT[:])
            ciT8_kv = ciT8_d[:].rearrange("g (k r) -> g k r", r=NQ * 128)
            for k in range(16):
                nc.sync.dma_start(
                    ciT8_kv[:, k, :], ciT[8 * k : 8 * k + 8, :]
                )
            # read view for pass-2 prep: [g, k, q, p]
            ciT8_rv = ciT8_d[:].rearrange("g (k q p) -> g k q p", q=NQ, p=128)

            # ---------- pass-1 helper ----------
            def p1_chunk_plan(nt_total):
                # 32-block chunks with a fine tail so the last stats chunks
                # drain through the pipeline quickly.
                plan = []
                t = 0
                while nt_total - t > 48:
                    plan.append((t, 32))
                    t += 32
                rem = nt_total - t
                for sz in (16, 8, 8, 4, 4, 2, 2, 1, 1, 1, 1, 1, 1, 1, 1):
                    if rem <= 0:
                        break
                    take = min(sz, rem)
                    plan.append((t, take))
                    t += take
                    rem -= take
                return plan

            def pass1(x_chunk, idx_sb, nt_total, ps, p1w, p1o, tail, off=0):
                """Segment sums into ps[32j+s] = [sum x | sum x^2 | count] for
                blocks t%4==j.  One 129-col matmul per block; rhs built by
                copy+square alternating between DVE and ACT per chunk."""
                tot = [0, 0, 0, 0]
                for t in range(nt_total):
                    tot[t % 4] += 1
                n_mm = [0, 0, 0, 0]
                pending = []

                def flush():
                    for t0, nb, rhs, oh in pending:
                        for k in range(nb):
                            j = (t0 + k) % 4
                            nc.tensor.matmul(
                                ps[32 * j : 32 * j + B, 0 : 2 * C + 1],
                                oh[:, k, :],
                                rhs[:, k, 0 : 2 * C + 1],
                                start=(n_mm[j] == 0),
                                stop=(n_mm[j] == tot[j] - 1),
                                tile_position=(0, 32 * j),
                                skip_group_check=True,
                            )
                            n_mm[j] += 1
                    pending.clear()

                plan = p1_chunk_plan(nt_total) if tail else list(
                    _chunks(nt_total, CH)
                )
                for ck, (t0, nb) in enumerate(plan):
                    x_ap = x_chunk(ck, t0, nb)
                    rhs = p1w.tile([128, CH, 132], BF16, tag="p1r")
                    if ck < MMB + 1:
                        nc.vector.memset(rhs[:, :, 2 * C : 2 * C + 1], 1.0)
                    if ck % 2 == 0:
                        nc.vector.tensor_copy(rhs[:, :nb, 0:C], x_ap)
                        nc.scalar.activation(
                            rhs[:, :nb, C : 2 * C],
                            x_ap,
                            mybir.ActivationFunctionType.Square,
                        )
                    else:
                        nc.scalar.activation(
                            rhs[:, :nb, 0:C],
                            x_ap,
                            mybir.ActivationFunctionType.Copy,
                        )
                        nc.vector.tensor_tensor(
                            rhs[:, :nb, C : 2 * C], x_ap, x_ap,
                            mybir.AluOpType.mult,
                        )
                    oh = p1o.tile([128, CH, B], BF16, tag="p1o")
                    nc.vector.tensor_tensor(
                        oh[:, :nb, :],
                        idx_sb[:, off + t0 : off + t0 + nb]
                        .unsqueeze(2)
                        .broadcast_to((128, nb, B)),
                        io16_sb[:].unsqueeze(1).broadcast_to((128, nb, B)),
                        mybir.AluOpType.is_equal,
                    )
                    pending.append((t0, nb, rhs, oh))
                    if len(pending) >= MMB:
                        flush()
                flush()

            def merge_stats(ps, psel, dst_sb):
                ev = constp.tile([128, 2 * C + 1], F32, tag="ev")
                nc.vector.memset(ev[:], 0.0)
                for j in range(4):
                    nc.scalar.copy(
                        ev[32 * j : 32 * j + B, :],
                        ps[32 * j : 32 * j + B, 0 : 2 * C + 1],
                    )
                nc.tensor.matmul(
                    psel[0:B, 0 : 2 * C + 1], s4_sb[:], ev[:], start=True,
                    stop=True,
                )
                nc.scalar.copy(dst_sb, psel[0:B, 0 : 2 * C + 1])

            def ar_start(src_sb, tag, w):
                inb = dramp.tile([B, w], F32, tag=f"in_{tag}")
                outb = dramp.tile([B, w], F32, tag=f"out_{tag}")
                nc.sync.dma_start(inb[:], src_sb)
                nc.gpsimd.collective_compute(
                    "AllReduce",
                    mybir.AluOpType.add,
                    replica_groups=[list(range(n_cores))],
                    ins=[inb.opt()],
                    outs=[outb.opt()],
                )
                return outb

            def seg_stats(g, mean_out, std_out):
                sums, ssq, cnt = g[:, 0:C], g[:, C : 2 * C], g[:, 2 * C : 2 * C + 1]
                rc_ = constp.tile([B, 1], F32, tag="t1")
                nc.vector.reciprocal(rc_[:], cnt)
                nm1 = constp.tile([B, 1], F32, tag="t2")
                nc.vector.tensor_scalar_add(nm1[:], cnt, -1.0)
                rnm1 = constp.tile([B, 1], F32, tag="t3")
                nc.vector.reciprocal(rnm1[:], nm1[:])
                fac = constp.tile([B, 1], F32, tag="t4")
                nc.vector.tensor_tensor(fac[:], cnt, rnm1[:], mybir.AluOpType.mult)
                nc.vector.tensor_scalar_mul(mean_out, sums, rc_[:])
                ex2 = constp.tile([B, C], F32, tag="t5")
                nc.vector.tensor_scalar_mul(ex2[:], ssq, rc_[:])
                m2 = constp.tile([B, C], F32, tag="t6")
                nc.scalar.square(m2[:], mean_out)
                var = constp.tile([B, C], F32, tag="t7")
                nc.vector.tensor_sub(var[:], ex2[:], m2[:])
                nc.vector.tensor_scalar_mul(var[:], var[:], fac[:])
                nc.vector.tensor_scalar_max(var[:], var[:], 0.0)
                nc.scalar.sqrt(std_out, var[:])
                nc.vector.tensor_scalar_add(std_out, std_out, EPS)

            gm_t = constp.tile([B, C], F32)
            gs_t = constp.tile([B, C], F32)
            stat2 = constp.tile([B, 2 * (2 * C + 1)], F32)

            with (
                tc.tile_pool(name="p1w", bufs=MMB + 1) as p1w,
                tc.tile_pool(name="p1o", bufs=MMB + 1) as p1o,
                tc.tile_pool(name="ps_p1", bufs=1, space="PSUM") as psp,
                tc.tile_pool(name="ps_sel", bufs=1, space="PSUM") as psel_p,
            ):
                # ---------- style pass 1 + early AR (hidden under content
                # stream-in) ----------
                ps_s = psp.tile([128, 512], F32, tag="ps_s")
                pass1(
                    lambda ck, t0, nb: sxcache[:, t0 : t0 + nb, :],
                    si_sb, nts, ps_s, p1w, p1o, False,
                )
                psel = psel_p.tile([128, 2 * C + 1], F32, tag="psel")
                merge_stats(ps_s, psel, stat2[:, 0 : 2 * C + 1])

                # ---------- content pass 1 ----------
                ps_c = psp.tile([128, 512], F32, tag="ps_c")
                pass1(
                    lambda ck, t0, nb: xcache[:, t0 : t0 + nb, :],
                    ci_sb, ntc, ps_c, p1w, p1o, True,
                )
                psel2 = psel_p.tile([128, 2 * C + 1], F32, tag="psel")
                merge_stats(ps_c, psel2, stat2[:, 2 * C + 1 :])
                outb_c = ar_start(stat2[:], "sc", w=2 * (2 * C + 1))

            # ---------- pass 2 ----------
            chunk_list = list(_chunks(ntc_pad, CH2))

            with (
                tc.tile_pool(name="p2ct", bufs=3) as p2ct,
                tc.tile_pool(name="p2oh", bufs=N_PRE + 2) as p2oh,
                tc.tile_pool(name="p2xb", bufs=2) as p2xb,
                tc.tile_pool(name="p2out", bufs=2) as p2out,
                tc.tile_pool(name="ps_b", bufs=2, space="PSUM") as psb_p,
                tc.tile_pool(name="ps_ga", bufs=2, space="PSUM") as psga_p,
                tc.tile_pool(name="ps_gb", bufs=2, space="PSUM") as psgb_p,
            ):
                def p2_prep(t0, nb):
                    """index slice load + K=8 broadcast MM + per-partition
                    compare -> transposed one-hot strips for nb blocks."""
                    ngr = nb // 8
                    w = ngr * 128
                    g0 = t0 // 8
                    q0, k0 = g0 // 16, g0 % 16
                    ct8 = p2ct.tile([8, (CH2 // 8) * 128], BF16, tag="ct8")
                    nc.sync.dma_start(
                        ct8[:, 0:w].rearrange("g (k p) -> g k p", p=128),
                        ciT8_rv[:, k0 : k0 + ngr, q0, :],
                    )
                    psB = psb_p.tile([128, 512], F32, tag="bc")
                    nc.tensor.matmul(
                        psB[:, 0:w], e8_sb[:], ct8[:, 0:w], start=True, stop=True
                    )
                    ohT = p2oh.tile([128, 512], BF16, tag="ohT")
                    nc.vector.tensor_scalar(
                        ohT[:, 0:w],
                        psB[:, 0:w],
                        pid_sb[:],
                        None,
                        mybir.AluOpType.is_equal,
                    )
                    return ohT

                # prep for the first chunks runs during the content AR
                preps = {}
                for ck in range(min(N_PRE, len(chunk_list))):
                    t0, nb = chunk_list[ck]
                    preps[ck] = p2_prep(t0, nb)

                # head-AR out -> style EMA math runs during the tail AR;
                # tail-AR out is then folded into the content stats.
                gstat2 = constp.tile([B, 2 * (2 * C + 1)], F32)
                nc.sync.dma_start(gstat2[:], outb_c[:])
                s_stats = constp.tile([B, 2 * C], F32)
                seg_stats(
                    gstat2[:, 0 : 2 * C + 1], s_stats[:, 0:C],
                    s_stats[:, C : 2 * C],
                )
                g_ps = psga_p.tile([128, 1024], F32, tag="ga")
                nc.tensor.matmul(
                    g_ps[0:B, 0 : 2 * C], el_sb[:], s_stats[:], start=True,
                    stop=True,
                )
                nc.vector.tensor_copy(gm_t[:], g_ps[0:B, 0:C])
                nc.vector.tensor_copy(gs_t[:], g_ps[0:B, C : 2 * C])
                # rgs = 1/g_std precomputed off the content-stats critical
                # path; then a = gs/std_c (depth 2) and b' = gm*std_c/gs -
                # mean_c (depth 3) run as independent short chains.
                rgs = constp.tile([B, C], F32)
                nc.vector.reciprocal(rgs[:], gs_t[:])
                gmrgs = constp.tile([B, C], F32)
                nc.vector.tensor_tensor(
                    gmrgs[:], gm_t[:], rgs[:], mybir.AluOpType.mult
                )
                mean_c = constp.tile([B, C], F32)
                std_c = constp.tile([B, C], F32)
                seg_stats(gstat2[:, 2 * C + 1 :], mean_c[:], std_c[:])
                rstd = constp.tile([B, C], F32)
                nc.vector.reciprocal(rstd[:], std_c[:])
                a_t = constp.tile([B, C], F32)
                nc.vector.tensor_tensor(
                    a_t[:], gs_t[:], rstd[:], mybir.AluOpType.mult
                )
                tmp = constp.tile([B, C], F32)
                nc.vector.tensor_tensor(
                    tmp[:], gmrgs[:], std_c[:], mybir.AluOpType.mult
                )
                bp_t = constp.tile([B, C], F32)
                nc.vector.tensor_sub(bp_t[:], tmp[:], mean_c[:])
                coef_a = constp.tile([B, C], BF16)
                nc.vector.tensor_copy(coef_a[:], a_t[:])
                coef_bp = constp.tile([B, C], BF16)
                nc.vector.tensor_copy(coef_bp[:], bp_t[:])
                # block-diagonal spread: row 16g+s holds coef[s] at cols
                # [64g, 64g+64); zero elsewhere (zero-filled early above).
                _q3 = [nc.sync, nc.scalar, nc.gpsimd]
                for g in range(8):
                    _q3[(2 * g) % 3].dma_start(
                        coef_bd_a[16 * g : 16 * g + B, C * g : C * g + C],
                        coef_a[:],
                    )
                    _q3[(2 * g + 1) % 3].dma_start(
                        coef_bd_b[16 * g : 16 * g + B, C * g : C * g + C],
                        coef_bp[:],
                    )

                # ---------- pass-2 main loop ----------
                # even groups: PE identity-MM accumulates x onto the b'
                # gather (PSUM), ACT evacs (x+b').  odd groups: b'-gather
                # only, ACT evacs b', GPSIMD adds x (spreads the +x between
                # the otherwise-idle GPSIMD and the PE).
                n_ga = 0
                for ck, (t0, nb) in enumerate(chunk_list):
                    ngr = nb // 8
                    ohT = preps[ck] if ck in preps else p2_prep(t0, nb)
                    ot = p2out.tile([128, CH2, C], BF16, tag="p2o")
                    for pair0 in range(0, ngr, 2):
                        pu = list(range(pair0, min(pair0 + 2, ngr)))
                        np_ = len(pu)
                        # psA holds both groups' a-gathers (one bank each);
                        # the pair shares one DVE mult and one xb pair tile.
                        psA = psga_p.tile([128, 1024], F32, tag="ga")
                        psBs = {}
                        kinds = {}
                        for i_u, u in enumerate(pu):
                            n_ga += 1
                            kinds[u] = n_ga % 2
                            psB2 = psgb_p.tile([128, 512], F32, tag="gb")
                            nc.tensor.matmul(
                                psA[:, 512 * i_u : 512 * (i_u + 1)],
                                ohT[:, u * 128 : (u + 1) * 128],
                                coef_bd_a[:],
                                start=True,
                                stop=True,
                                skip_group_check=True,
                            )
                            nc.tensor.matmul(
                                psB2[:],
                                ohT[:, u * 128 : (u + 1) * 128],
                                coef_bd_b[:],
                                start=True,
                                stop=(kinds[u] == 1),
                                skip_group_check=True,
                            )
                            psBs[u] = psB2
                        # identity x-accumulates for even groups only
                        for u in pu:
                            if kinds[u] == 1:
                                continue
                            b0 = t0 + 8 * u
                            nc.tensor.matmul(
                                psBs[u][:],
                                ident[:],
                                xcache[:, b0 : b0 + 8, :].rearrange(
                                    "p n d -> p (n d)"
                                ),
                                start=False,
                                stop=True,
                                skip_group_check=True,
                            )
                        xbp = p2xb.tile([128, 16, C], BF16, tag="xb")
                        for i_u, u in enumerate(pu):
                            b0 = t0 + 8 * u
                            if kinds[u] == 1:
                                xb0 = p2xb.tile([128, 8, C], BF16, tag="xb0")
                                nc.scalar.copy(
                                    xb0[:],
                                    psBs[u][:].rearrange(
                                        "p (n d) -> p n d", d=C
                                    ),
                                )
                                nc.gpsimd.tensor_tensor(
                                    xbp[:, 8 * i_u : 8 * i_u + 8, :],
                                    xb0[:],
                                    xcache[:, b0 : b0 + 8, :],
                                    mybir.AluOpType.add,
                                )
                            else:
                                nc.scalar.copy(
                                    xbp[:, 8 * i_u : 8 * i_u + 8, :],
                                    psBs[u][:].rearrange(
                                        "p (n d) -> p n d", d=C
                                    ),
                                )
                        nc.vector.tensor_tensor(
                            ot[:, 8 * pair0 : 8 * (pair0 + np_), :],
                            psA[:, 0 : 512 * np_].rearrange(
                                "p (n d) -> p n d", d=C
                            ),
                            xbp[:, 0 : 8 * np_, :],
                            mybir.AluOpType.mult,
                        )
                    nreal = min(nb, ntc - t0)
                    if nreal > 0:
                        out_q(ck).dma_start(
                            out_v[:, t0 : t0 + nreal, :], ot[:, :nreal, :]
                        )

    nc.compile()
    return nc


_NC_CACHE = {}


def _get_nc(rc=RC, rs=RS, n_cores=N_CORES):
    key = (rc, rs, n_cores)
    if key not in _NC_CACHE:
        _NC_CACHE[key] = build_nc(rc, rs, n_cores)
    return _NC_CACHE[key]


def _pad_rows(a: np.ndarray, total: int, fill) -> np.ndarray:
    pad = total - a.shape[0]
    if pad == 0:
        return np.ascontiguousarray(a)
    pad_shape = (pad,) + a.shape[1:]
    return np.concatenate([a, np.full(pad_shape, fill, a.dtype)], axis=0)


def make_in_maps(cf, ci, sf, si, rc=RC, rs=RS, n_cores=N_CORES):
    cf = _pad_rows(np.asarray(cf, np.float32), n_cores * rc, 0.0)
    ci = _pad_rows(np.asarray(ci, np.int32), n_cores * rc, B)
    sf = _pad_rows(np.asarray(sf, np.float32), n_cores * rs, 0.0)
    si = _pad_rows(np.asarray(si, np.int32), n_cores * rs, B)
    el = _ema_lhsT()
    e8 = _e8()
    s4 = _s4()
    io16 = _io16()
    pid16 = _pid16()
    return [
        {
            "cf": np.ascontiguousarray(cf[k * rc : (k + 1) * rc]),
            "ci": np.ascontiguousarray(ci[k * rc : (k + 1) * rc]),
            "sf": np.ascontiguousarray(sf[k * rs : (k + 1) * rs]),
            "si": np.ascontiguousarray(si[k * rs : (k + 1) * rs]),
            "el": el,
            "e8": e8,
            "s4": s4,
            "io16": io16,
            "pid16": pid16,
        }
        for k in range(n_cores)
    ]


def kernel(
    content_feats: np.ndarray,
    style_feats: np.ndarray,
    content_batch_indices: np.ndarray,
    style_batch_indices: np.ndarray,
    num_batches=B,
) -> np.ndarray:
    n_c = content_feats.shape[0]
    nc = _get_nc()
    in_maps = make_in_maps(
        content_feats, content_batch_indices, style_feats, style_batch_indices
    )
    res = bass_utils.run_bass_kernel_spmd(nc, in_maps, core_ids=list(range(N_CORES)))
    out = np.concatenate(
        [np.asarray(res.results[k]["out"]) for k in range(N_CORES)], axis=0
    )
    return np.ascontiguousarray(out[:n_c]).astype(np.float32)


# revision 35
# speedup vs baseline: 1.0816x; 1.0816x over previous
"""AdaIN (segment mean/std + EMA of style stats) on 8 TRN2 NeuronCores — v5.

vs v4 baseline (347us -> ~296us):
  - pass-2 FMA re-associated as out = a (.) (x + b/a): the b'-gather
    matmul PSUM-accumulates an identity-matmul copy of x, so per row the
    elementwise work is one ACT evac (x+b' -> bf16) + one DVE mult
    (a_psum * evac -> bf16 out) instead of evac + mult + add.  For every
    other gather group the +x runs as a GPSIMD add on the evac'd b'
    instead of the identity matmul (spreads work onto the idle engine).
  - pair-granularity apply: two groups share one [128,1024] a-gather PSUM
    tile, one xb pair tile, and a single FD-1024 DVE mult / pair.
  - one-hot strips via per-partition tensor_scalar is_equal straight from
    the broadcast-matmul PSUM (no separate iota broadcast tile).
  - ciT8 regroup DMA layout is k-major so each of the 16 transposed-index
    regroup writes is a contiguous 2KB-per-partition HBM write (the
    q-major layout wrote 256B runs that crawled ~90us and, via a pool
    barrier, stalled the whole kernel).
  - identity/iota consts are built before any bulk-load dma_start is
    emitted (the gpsimd queue paces with the DMA pipeline, so anything
    behind the loads starts ~40us late).
  - a dummy AllReduce triggered before the bulk stream absorbs the
    first-collective warmup; the single combined stats AR at the end of
    pass 1 then completes in ~10-20us, with N_PRE pass-2 preps hoisted
    into its latency window.
  - pass-1 rhs builds alternate engines per chunk (DVE copy + ACT square
    / ACT copy + DVE square) — both run at 1-2x, and alternation keeps
    either engine from pacing the stream; content loads are 64-block
    grained with a fine-grained tail and the pass-1 chunk plan tapers so
    the stats tail drains quickly into the AR.
"""

import os
import sys

import numpy as np

for _p in ("/opt/trn_rl_repo",):
    if _p not in sys.path and os.path.isdir(_p):
        sys.path.insert(0, _p)

from concourse import bacc, bass, bass_utils, masks, mybir, tile

F32 = mybir.dt.float32
BF16 = mybir.dt.bfloat16
I32 = mybir.dt.int32
I16 = mybir.dt.int16

N_CORES = 8
C = 64
B = 16
ALPHA = 0.1
EPS = 1e-8

RC = 128 * 977  # per-core content rows (125056; 8*RC >= 1M)
RS = 128 * 245  # per-core style rows (31360; 8*RS >= 250K)

CH = 32    # pass-1 chunk: blocks per onehot/square tile
MMB = 2    # pass-1 chunks per matmul burst
CH2 = 32   # pass-2 chunk: blocks per out tile (4 gathers of 8)
N_PRE = 9  # pass-2 chunks whose prep is hoisted into the content-AR window


def _ema_lhsT() -> np.ndarray:
    L = np.zeros((B, B), np.float64)
    for b in range(B):
        L[b, 0] = (1.0 - ALPHA) ** b
        for j in range(1, b + 1):
            L[b, j] = ALPHA * (1.0 - ALPHA) ** (b - j)
    return np.ascontiguousarray(L.T).astype(np.float32)


def _e8() -> np.ndarray:
    # e8[g, 16g+s] = 1 : broadcast row g of a K=8 rhs to partitions 16g..16g+16
    e = np.zeros((8, 128), np.float32)
    for g in range(8):
        e[g, 16 * g : 16 * g + 16] = 1.0
    return e


def _s4() -> np.ndarray:
    # s4[32j+s, s] = 1 (s<16): sum the 4 col-group partial stats
    s = np.zeros((128, B), np.float32)
    for j in range(4):
        for t in range(B):
            s[32 * j + t, t] = 1.0
    return s


def _io16() -> np.ndarray:
    return np.broadcast_to(np.arange(B, dtype=np.int16), (128, B)).copy()


def _pid16() -> np.ndarray:
    return (np.arange(128, dtype=np.float32) % 16).reshape(128, 1)


def _chunks(total: int, step: int):
    t0 = 0
    while t0 < total:
        yield t0, min(step, total - t0)
        t0 += step


def _load_plan(ntc: int):
    # 64-block (2MB) loads with a fine-grained tail so the last pass-1
    # chunks aren't gated on one huge DMA completion.
    plan = []
    t = 0
    while ntc - t > 80:
        plan.append((t, 64))
        t += 64
    rem = ntc - t
    for sz in (32, 16, 8, 8, 4, 4, 2, 2, 1, 1):
        if rem <= 0:
            break
        take = min(sz, rem)
        plan.append((t, take))
        t += take
        rem -= take
    while rem > 0:
        plan.append((t, 1))
        t += 1
        rem -= 1
    return plan


def build_nc(rc: int = RC, rs: int = RS, n_cores: int = N_CORES):
    ntc = rc // 128
    nts = rs // 128
    ntc_pad = ((ntc + 7) // 8) * 8
    ntc_t = ((ntc + 127) // 128) * 128

    nc = bacc.Bacc(
        "TRN2", target_bir_lowering=False, debug=False, num_devices=n_cores
    )
    cf = nc.dram_tensor("cf", [rc, C], F32, kind="ExternalInput")
    ci = nc.dram_tensor("ci", [rc], I32, kind="ExternalInput")
    sf = nc.dram_tensor("sf", [rs, C], F32, kind="ExternalInput")
    si = nc.dram_tensor("si", [rs], I32, kind="ExternalInput")
    el = nc.dram_tensor("el", [B, B], F32, kind="ExternalInput")
    e8 = nc.dram_tensor("e8", [8, 128], F32, kind="ExternalInput")
    s4 = nc.dram_tensor("s4", [128, B], F32, kind="ExternalInput")
    io16 = nc.dram_tensor("io16", [128, B], I16, kind="ExternalInput")
    pid16 = nc.dram_tensor("pid16", [128, 1], F32, kind="ExternalInput")
    out = nc.dram_tensor("out", [rc, C], BF16, kind="ExternalOutput")

    cf_v = cf.ap().rearrange("(p n) d -> p n d", p=128)
    ci_v = ci.ap().rearrange("(p n) -> p n", p=128)
    sf_v = sf.ap().rearrange("(p n) d -> p n d", p=128)
    si_v = si.ap().rearrange("(p n) -> p n", p=128)
    out_v = out.ap().rearrange("(p n) d -> p n d", p=128)

    def out_q(i):
        return nc.sync if i % 2 == 0 else nc.scalar

    with tile.TileContext(nc) as tc:
        with (
            tc.tile_pool(name="const", bufs=1) as constp,
            tc.tile_pool(name="cache", bufs=1) as cachep,
            tc.tile_pool(name="dram", bufs=1, space="DRAM") as dramp,
        ):
            # identity built FIRST: its gpsimd ops must not queue behind the
            # bulk-load dma_starts (each paces with the DMA pipeline).
            ident = constp.tile([128, 128], BF16)
            masks.make_identity(nc, ident[:])
            ones_sb = constp.tile([128, 1], BF16)
            nc.gpsimd.memset(ones_sb[:], 1.0)

            # ---------- bulk loads, all on the gpsimd SWDGE queue so the
            # descriptor order controls arrival: indices first, then style,
            # then content.  Feature loads are f32->bf16 cast-DMAs into
            # resident bf16 caches. ----------
            ci_sb = constp.tile([128, ntc], I32)
            nc.gpsimd.dma_start(ci_sb[:], ci_v)
            si_sb = constp.tile([128, nts], I32)
            nc.gpsimd.dma_start(si_sb[:], si_v)
            # dummy AllReduce, triggered before the bulk stream: absorbs the
            # first-collective warmup so the real AR at the end runs ~10us.
            din = dramp.tile([B, B], F32, tag="din")
            dout = dramp.tile([B, B], F32, tag="dout")
            nc.sync.dma_start(din[:], el.ap())
            nc.gpsimd.collective_compute(
                "AllReduce",
                mybir.AluOpType.add,
                replica_groups=[list(range(n_cores))],
                ins=[din.opt()],
                outs=[dout.opt()],
            )
            sxcache = cachep.tile([128, nts, C], BF16)
            for l0, nl in _chunks(nts, 64):
                nc.gpsimd.dma_start(
                    sxcache[:, l0 : l0 + nl, :], sf_v[:, l0 : l0 + nl, :]
                )
            # content loads are split around the style-AR trigger (the
            # collective rides the gpsimd queue): loads emitted before it
            # keep the SDMA engines fed while the trigger waits.
            xcache = cachep.tile([128, ntc_pad, C], BF16)
            load_list = _load_plan(ntc)

            def emit_content_loads(lo, hi):
                for l0, nl in load_list[lo:hi]:
                    nc.gpsimd.dma_start(
                        xcache[:, l0 : l0 + nl, :], cf_v[:, l0 : l0 + nl, :]
                    )

            emit_content_loads(0, None)

            # ---------- constants ----------
            el_sb = constp.tile([B, B], F32)
            nc.sync.dma_start(el_sb[:], el.ap())
            e8_sbf = constp.tile([8, 128], F32)
            nc.sync.dma_start(e8_sbf[:], e8.ap())
            e8_sb = constp.tile([8, 128], BF16)
            nc.vector.tensor_copy(e8_sb[:], e8_sbf[:])
            s4_sb = constp.tile([128, B], F32)
            nc.sync.dma_start(s4_sb[:], s4.ap())
            io16_sb = constp.tile([128, B], I16)
            nc.sync.dma_start(io16_sb[:], io16.ap())
            pid_sb = constp.tile([128, 1], F32)
            nc.sync.dma_start(pid_sb[:], pid16.ap())

            if ntc_pad > ntc:
                nc.vector.memset(xcache[:, ntc:ntc_pad, :], 0.0)

            # coef_bd zero-fill happens early (off the post-AR critical path)
            coef_bd_a = constp.tile([128, 8 * C], BF16)
            coef_bd_b = constp.tile([128, 8 * C], BF16)
            nc.vector.memset(coef_bd_a[:], 0.0)
            nc.vector.memset(coef_bd_b[:], 0.0)

            # ---------- transposed indices -> ciT8 in DRAM, k-major layout:
            # ciT8[g, (k*Q+q)*128 + p] = ciT[8k+g, 128q+p] = idx of lane p in
            # block 128q+8k+g.  k-major makes each regroup DMA a contiguous
            # 2KB-per-partition write (the q-major layout wrote 256B runs,
            # which crawl on HBM). ----------
            NQ = ntc_t // 128
            idxbf = constp.tile([128, ntc_t], BF16)
            if ntc_t > ntc:
                nc.vector.memset(idxbf[:, ntc:ntc_t], float(B))
            nc.vector.tensor_copy(idxbf[:, 0:ntc], ci_sb[:])
            ciT = constp.tile([128, ntc_t], BF16)
            ciT8_d = dramp.tile([8, 16 * NQ * 128], BF16, tag="ciT8")
            with tc.tile_pool(name="ps_tr", bufs=2, space="PSUM") as pstr:
                for q in range(NQ):
                    psT = pstr.tile([128, 128], BF16, tag="tr")
                    nc.tensor.transpose(
                        psT[:], idxbf[:, 128 * q : 128 * (q + 1)], ident[:]
                    )
                    nc.scalar.copy(ciT[:, 128 * q : 128 * (q + 1)], psT[:])
            ciT8_kv = ciT8_d[:].rearrange("g (k r) -> g k r", r=NQ * 128)
            for k in range(16):
                nc.sync.dma_start(
                    ciT8_kv[:, k, :], ciT[8 * k : 8 * k + 8, :]
                )
            # read view for pass-2 prep: [g, k, q, p]
            ciT8_rv = ciT8_d[:].rearrange("g (k q p) -> g k q p", q=NQ, p=128)

            # ---------- pass-1 helper ----------
            def p1_chunk_plan(nt_total):
                # 32-block chunks with a fine tail so the last stats chunks
                # drain through the pipeline quickly.
                plan = []
                t = 0
                while nt_total - t > 48:
                    plan.append((t, 32))
                    t += 32
                rem = nt_total - t
                for sz in (16, 8, 8, 4, 4, 2, 2, 1, 1, 1, 1, 1, 1, 1, 1):
                    if rem <= 0:
                        break
                    take = min(sz, rem)
                    plan.append((t, take))
                    t += take
                    rem -= take
                return plan

            def pass1(x_chunk, idx_sb, nt_total, ps, p1w, p1o, tail, off=0):
                """Segment sums into ps[32j+s] = [sum x | sum x^2 | count] for
                blocks t%4==j.  One 129-col matmul per block; rhs built by
                copy+square alternating between DVE and ACT per chunk."""
                tot = [0, 0, 0, 0]
                for t in range(nt_total):
                    tot[t % 4] += 1
                n_mm = [0, 0, 0, 0]
                pending = []

                def flush():
                    for t0, nb, rhs, oh in pending:
                        for k in range(nb):
                            j = (t0 + k) % 4
                            nc.tensor.matmul(
                                ps[32 * j : 32 * j + B, 0 : 2 * C + 1],
                                oh[:, k, :],
                                rhs[:, k, 0 : 2 * C + 1],
                                start=(n_mm[j] == 0),
                                stop=(n_mm[j] == tot[j] - 1),
                                tile_position=(0, 32 * j),
                                skip_group_check=True,
                            )
                            n_mm[j] += 1
                    pending.clear()

                plan = p1_chunk_plan(nt_total) if tail else list(
                    _chunks(nt_total, CH)
                )
                for ck, (t0, nb) in enumerate(plan):
                    x_ap = x_chunk(ck, t0, nb)
                    rhs = p1w.tile([128, CH, 132], BF16, tag="p1r")
                    if ck < MMB + 1:
                        nc.vector.memset(rhs[:, :, 2 * C : 2 * C + 1], 1.0)
                    if ck % 2 == 0:
                        nc.vector.tensor_copy(rhs[:, :nb, 0:C], x_ap)
                        nc.scalar.activation(
                            rhs[:, :nb, C : 2 * C],
                            x_ap,
                            mybir.ActivationFunctionType.Square,
                        )
                    else:
                        nc.scalar.activation(
                            rhs[:, :nb, 0:C],
                            x_ap,
                            mybir.ActivationFunctionType.Copy,
                        )
                        nc.vector.tensor_tensor(
                            rhs[:, :nb, C : 2 * C], x_ap, x_ap,
                            mybir.AluOpType.mult,
                        )
                    oh = p1o.tile([128, CH, B], BF16, tag="p1o")
                    nc.vector.tensor_tensor(
                        oh[:, :nb, :],
                        idx_sb[:, off + t0 : off + t0 + nb]
                        .unsqueeze(2)
                        .broadcast_to((128, nb, B)),
                        io16_sb[:].unsqueeze(1).broadcast_to((128, nb, B)),
                        mybir.AluOpType.is_equal,
                    )
                    pending.append((t0, nb, rhs, oh))
                    if len(pending) >= MMB:
                        flush()
                flush()

            def merge_stats(ps, psel, dst_sb):
                ev = constp.tile([128, 2 * C + 1], F32, tag="ev")
                nc.vector.memset(ev[:], 0.0)
                for j in range(4):
                    nc.scalar.copy(
                        ev[32 * j : 32 * j + B, :],
                        ps[32 * j : 32 * j + B, 0 : 2 * C + 1],
                    )
                nc.tensor.matmul(
                    psel[0:B, 0 : 2 * C + 1], s4_sb[:], ev[:], start=True,
                    stop=True,
                )
                nc.scalar.copy(dst_sb, psel[0:B, 0 : 2 * C + 1])

            def ar_start(src_sb, tag, w):
                inb = dramp.tile([B, w], F32, tag=f"in_{tag}")
                outb = dramp.tile([B, w], F32, tag=f"out_{tag}")
                nc.sync.dma_start(inb[:], src_sb)
                nc.gpsimd.collective_compute(
                    "AllReduce",
                    mybir.AluOpType.add,
                    replica_groups=[list(range(n_cores))],
                    ins=[inb.opt()],
                    outs=[outb.opt()],
                )
                return outb

            def seg_stats(g, mean_out, std_out):
                sums, ssq, cnt = g[:, 0:C], g[:, C : 2 * C], g[:, 2 * C : 2 * C + 1]
                rc_ = constp.tile([B, 1], F32, tag="t1")
                nc.vector.reciprocal(rc_[:], cnt)
                nm1 = constp.tile([B, 1], F32, tag="t2")
                nc.vector.tensor_scalar_add(nm1[:], cnt, -1.0)
                rnm1 = constp.tile([B, 1], F32, tag="t3")
                nc.vector.reciprocal(rnm1[:], nm1[:])
                fac = constp.tile([B, 1], F32, tag="t4")
                nc.vector.tensor_tensor(fac[:], cnt, rnm1[:], mybir.AluOpType.mult)
                nc.vector.tensor_scalar_mul(mean_out, sums, rc_[:])
                ex2 = constp.tile([B, C], F32, tag="t5")
                nc.vector.tensor_scalar_mul(ex2[:], ssq, rc_[:])
                m2 = constp.tile([B, C], F32, tag="t6")
                nc.scalar.square(m2[:], mean_out)
                var = constp.tile([B, C], F32, tag="t7")
                nc.vector.tensor_sub(var[:], ex2[:], m2[:])
                nc.vector.tensor_scalar_mul(var[:], var[:], fac[:])
                nc.vector.tensor_scalar_max(var[:], var[:], 0.0)
                nc.scalar.sqrt(std_out, var[:])
                nc.vector.tensor_scalar_add(std_out, std_out, EPS)

            gm_t = constp.tile([B, C], F32)
            gs_t = constp.tile([B, C], F32)
            stat2 = constp.tile([B, 2 * (2 * C + 1)], F32)

            with (
                tc.tile_pool(name="p1w", bufs=MMB + 1) as p1w,
                tc.tile_pool(name="p1o", bufs=MMB + 1) as p1o,
                tc.tile_pool(name="ps_p1", bufs=1, space="PSUM") as psp,
                tc.tile_pool(name="ps_sel", bufs=1, space="PSUM") as psel_p,
            ):
                # ---------- style pass 1 + early AR (hidden under content
                # stream-in) ----------
                ps_s = psp.tile([128, 512], F32, tag="ps_s")
                pass1(
                    lambda ck, t0, nb: sxcache[:, t0 : t0 + nb, :],
                    si_sb, nts, ps_s, p1w, p1o, False,
                )
                psel = psel_p.tile([128, 2 * C + 1], F32, tag="psel")
                merge_stats(ps_s, psel, stat2[:, 0 : 2 * C + 1])

                # ---------- content pass 1 ----------
                ps_c = psp.tile([128, 512], F32, tag="ps_c")
                pass1(
                    lambda ck, t0, nb: xcache[:, t0 : t0 + nb, :],
                    ci_sb, ntc, ps_c, p1w, p1o, True,
                )
                psel2 = psel_p.tile([128, 2 * C + 1], F32, tag="psel")
                merge_stats(ps_c, psel2, stat2[:, 2 * C + 1 :])
                outb_c = ar_start(stat2[:], "sc", w=2 * (2 * C + 1))

            # ---------- pass 2 ----------
            chunk_list = list(_chunks(ntc_pad, CH2))

            with (
                tc.tile_pool(name="p2ct", bufs=3) as p2ct,
                tc.tile_pool(name="p2oh", bufs=N_PRE + 2) as p2oh,
                tc.tile_pool(name="p2xb", bufs=2) as p2xb,
                tc.tile_pool(name="p2out", bufs=2) as p2out,
                tc.tile_pool(name="ps_b", bufs=2, space="PSUM") as psb_p,
                tc.tile_pool(name="ps_ga", bufs=2, space="PSUM") as psga_p,
                tc.tile_pool(name="ps_gb", bufs=2, space="PSUM") as psgb_p,
            ):
                def p2_prep(t0, nb):
                    """index slice load + K=8 broadcast MM + per-partition
                    compare -> transposed one-hot strips for nb blocks."""
                    ngr = nb // 8
                    w = ngr * 128
                    g0 = t0 // 8
                    q0, k0 = g0 // 16, g0 % 16
                    ct8 = p2ct.tile([8, (CH2 // 8) * 128], BF16, tag="ct8")
                    nc.sync.dma_start(
                        ct8[:, 0:w].rearrange("g (k p) -> g k p", p=128),
                        ciT8_rv[:, k0 : k0 + ngr, q0, :],
                    )
                    psB = psb_p.tile([128, 512], F32, tag="bc")
                    nc.tensor.matmul(
                        psB[:, 0:w], e8_sb[:], ct8[:, 0:w], start=True, stop=True
                    )
                    ohT = p2oh.tile([128, 512], BF16, tag="ohT")
                    nc.vector.tensor_scalar(
                        ohT[:, 0:w],
                        psB[:, 0:w],
                        pid_sb[:],
                        None,
                        mybir.AluOpType.is_equal,
                    )
                    return ohT

                # prep for the first chunks runs during the content AR
                preps = {}
                for ck in range(min(N_PRE, len(chunk_list))):
                    t0, nb = chunk_list[ck]
                    preps[ck] = p2_prep(t0, nb)

                # head-AR out -> style EMA math runs during the tail AR;
                # tail-AR out is then folded into the content stats.
                gstat2 = constp.tile([B, 2 * (2 * C + 1)], F32)
                nc.sync.dma_start(gstat2[:], outb_c[:])
                s_stats = constp.tile([B, 2 * C], F32)
                seg_stats(
                    gstat2[:, 0 : 2 * C + 1], s_stats[:, 0:C],
                    s_stats[:, C : 2 * C],
                )
                g_ps = psga_p.tile([128, 1024], F32, tag="ga")
                nc.tensor.matmul(
                    g_ps[0:B, 0 : 2 * C], el_sb[:], s_stats[:], start=True,
                    stop=True,
                )
                nc.vector.tensor_copy(gm_t[:], g_ps[0:B, 0:C])
                nc.vector.tensor_copy(gs_t[:], g_ps[0:B, C : 2 * C])
                mean_c = constp.tile([B, C], F32)
                std_c = constp.tile([B, C], F32)
                seg_stats(gstat2[:, 2 * C + 1 :], mean_c[:], std_c[:])
                rstd = constp.tile([B, C], F32)
                nc.vector.reciprocal(rstd[:], std_c[:])
                a_t = constp.tile([B, C], F32)
                nc.vector.tensor_tensor(
                    a_t[:], gs_t[:], rstd[:], mybir.AluOpType.mult
                )
                tmp = constp.tile([B, C], F32)
                nc.vector.tensor_tensor(
                    tmp[:], mean_c[:], a_t[:], mybir.AluOpType.mult
                )
                b_t = constp.tile([B, C], F32)
                nc.vector.tensor_sub(b_t[:], gm_t[:], tmp[:])
                ra_t = constp.tile([B, C], F32)
                nc.vector.reciprocal(ra_t[:], a_t[:])
                bp_t = constp.tile([B, C], F32)
                nc.vector.tensor_tensor(
                    bp_t[:], b_t[:], ra_t[:], mybir.AluOpType.mult
                )
                coef_a = constp.tile([B, C], BF16)
                nc.vector.tensor_copy(coef_a[:], a_t[:])
                coef_bp = constp.tile([B, C], BF16)
                nc.vector.tensor_copy(coef_bp[:], bp_t[:])
                # block-diagonal spread: row 16g+s holds coef[s] at cols
                # [64g, 64g+64); zero elsewhere (zero-filled early above).
                _q3 = [nc.sync, nc.scalar, nc.gpsimd]
                for g in range(8):
                    _q3[(2 * g) % 3].dma_start(
                        coef_bd_a[16 * g : 16 * g + B, C * g : C * g + C],
                        coef_a[:],
                    )
                    _q3[(2 * g + 1) % 3].dma_start(
                        coef_bd_b[16 * g : 16 * g + B, C * g : C * g + C],
                        coef_bp[:],
                    )

                # ---------- pass-2 main loop ----------
                # even groups: PE identity-MM accumulates x onto the b'
                # gather (PSUM), ACT evacs (x+b').  odd groups: b'-gather
                # only, ACT evacs b', GPSIMD adds x (spreads the +x between
                # the otherwise-idle GPSIMD and the PE).
                n_ga = 0
                for ck, (t0, nb) in enumerate(chunk_list):
                    ngr = nb // 8
                    ohT = preps[ck] if ck in preps else p2_prep(t0, nb)
                    ot = p2out.tile([128, CH2, C], BF16, tag="p2o")
                    for pair0 in range(0, ngr, 2):
                        pu = list(range(pair0, min(pair0 + 2, ngr)))
                        np_ = len(pu)
                        # psA holds both groups' a-gathers (one bank each);
                        # the pair shares one DVE mult and one xb pair tile.
                        psA = psga_p.tile([128, 1024], F32, tag="ga")
                        psBs = {}
                        kinds = {}
                        for i_u, u in enumerate(pu):
                            n_ga += 1
                            kinds[u] = n_ga % 2
                            psB2 = psgb_p.tile([128, 512], F32, tag="gb")
                            nc.tensor.matmul(
                                psA[:, 512 * i_u : 512 * (i_u + 1)],
                                ohT[:, u * 128 : (u + 1) * 128],
                                coef_bd_a[:],
                                start=True,
                                stop=True,
                                skip_group_check=True,
                            )
                            nc.tensor.matmul(
                                psB2[:],
                                ohT[:, u * 128 : (u + 1) * 128],
                                coef_bd_b[:],
                                start=True,
                                stop=(kinds[u] == 1),
                                skip_group_check=True,
                            )
                            psBs[u] = psB2
                        # identity x-accumulates for even groups only
                        for u in pu:
                            if kinds[u] == 1:
                                continue
                            b0 = t0 + 8 * u
                            nc.tensor.matmul(
                                psBs[u][:],
                                ident[:],
                                xcache[:, b0 : b0 + 8, :].rearrange(
                                    "p n d -> p (n d)"
                                ),
                                start=False,
                                stop=True,
                                skip_group_check=True,
                            )
                        xbp = p2xb.tile([128, 16, C], BF16, tag="xb")
                        for i_u, u in enumerate(pu):
                            b0 = t0 + 8 * u
                            if kinds[u] == 1:
                                xb0 = p2xb.tile([128, 8, C], BF16, tag="xb0")
                                nc.scalar.copy(
                                    xb0[:],
                                    psBs[u][:].rearrange(
                                        "p (n d) -> p n d", d=C
                                    ),
                                )
                                nc.gpsimd.tensor_tensor(
                                    xbp[:, 8 * i_u : 8 * i_u + 8, :],
                                    xb0[:],
                                    xcache[:, b0 : b0 + 8, :],
                                    mybir.AluOpType.add,
                                )
                            else:
                                nc.scalar.copy(
                                    xbp[:, 8 * i_u : 8 * i_u + 8, :],
                                    psBs[u][:].rearrange(
                                        "p (n d) -> p n d", d=C
                                    ),
                                )
                        nc.vector.tensor_tensor(
                            ot[:, 8 * pair0 : 8 * (pair0 + np_), :],
                            psA[:, 0 : 512 * np_].rearrange(
                                "p (n d) -> p n d", d=C
                            ),
                            xbp[:, 0 : 8 * np_, :],
                            mybir.AluOpType.mult,
                        )
                    nreal = min(nb, ntc - t0)
                    if nreal > 0:
                        out_q(ck).dma_start(
                            out_v[:, t0 : t0 + nreal, :], ot[:, :nreal, :]
                        )

    nc.compile()
    return nc


_NC_CACHE = {}


def _get_nc(rc=RC, rs=RS, n_cores=N_CORES):
    key = (rc, rs, n_cores)
    if key not in _NC_CACHE:
        _NC_CACHE[key] = build_nc(rc, rs, n_cores)
    return _NC_CACHE[key]


def _pad_rows(a: np.ndarray, total: int, fill) -> np.ndarray:
    pad = total - a.shape[0]
    if pad == 0:
        return np.ascontiguousarray(a)
    pad_shape = (pad,) + a.shape[1:]
    return np.concatenate([a, np.full(pad_shape, fill, a.dtype)], axis=0)


def make_in_maps(cf, ci, sf, si, rc=RC, rs=RS, n_cores=N_CORES):
    cf = _pad_rows(np.asarray(cf, np.float32), n_cores * rc, 0.0)
    ci = _pad_rows(np.asarray(ci, np.int32), n_cores * rc, B)
    sf = _pad_rows(np.asarray(sf, np.float32), n_cores * rs, 0.0)
    si = _pad_rows(np.asarray(si, np.int32), n_cores * rs, B)
    el = _ema_lhsT()
    e8 = _e8()
    s4 = _s4()
    io16 = _io16()
    pid16 = _pid16()
    return [
        {
            "cf": np.ascontiguousarray(cf[k * rc : (k + 1) * rc]),
            "ci": np.ascontiguousarray(ci[k * rc : (k + 1) * rc]),
            "sf": np.ascontiguousarray(sf[k * rs : (k + 1) * rs]),
            "si": np.ascontiguousarray(si[k * rs : (k + 1) * rs]),
            "el": el,
            "e8": e8,
            "s4": s4,
            "io16": io16,
            "pid16": pid16,
        }
        for k in range(n_cores)
    ]


def kernel(
    content_feats: np.ndarray,
    style_feats: np.ndarray,
    content_batch_indices: np.ndarray,
    style_batch_indices: np.ndarray,
    num_batches=B,
) -> np.ndarray:
    n_c = content_feats.shape[0]
    nc = _get_nc()
    in_maps = make_in_maps(
        content_feats, content_batch_indices, style_feats, style_batch_indices
    )
    res = bass_utils.run_bass_kernel_spmd(nc, in_maps, core_ids=list(range(N_CORES)))
    out = np.concatenate(
        [np.asarray(res.results[k]["out"]) for k in range(N_CORES)], axis=0
    )
    return np.ascontiguousarray(out[:n_c]).astype(np.float32)


# revision 36
# speedup vs baseline: 1.0913x; 1.0090x over previous
"""AdaIN (segment mean/std + EMA of style stats) on 8 TRN2 NeuronCores — v5.

vs v4 baseline (347us -> ~296us):
  - pass-2 FMA re-associated as out = a (.) (x + b/a): the b'-gather
    matmul PSUM-accumulates an identity-matmul copy of x, so per row the
    elementwise work is one ACT evac (x+b' -> bf16) + one DVE mult
    (a_psum * evac -> bf16 out) instead of evac + mult + add.  For every
    other gather group the +x runs as a GPSIMD add on the evac'd b'
    instead of the identity matmul (spreads work onto the idle engine).
  - pair-granularity apply: two groups share one [128,1024] a-gather PSUM
    tile, one xb pair tile, and a single FD-1024 DVE mult / pair.
  - one-hot strips via per-partition tensor_scalar is_equal straight from
    the broadcast-matmul PSUM (no separate iota broadcast tile).
  - ciT8 regroup DMA layout is k-major so each of the 16 transposed-index
    regroup writes is a contiguous 2KB-per-partition HBM write (the
    q-major layout wrote 256B runs that crawled ~90us and, via a pool
    barrier, stalled the whole kernel).
  - identity/iota consts are built before any bulk-load dma_start is
    emitted (the gpsimd queue paces with the DMA pipeline, so anything
    behind the loads starts ~40us late).
  - a dummy AllReduce triggered before the bulk stream absorbs the
    first-collective warmup; the single combined stats AR at the end of
    pass 1 then completes in ~10-20us, with N_PRE pass-2 preps hoisted
    into its latency window.
  - pass-1 rhs builds alternate engines per chunk (DVE copy + ACT square
    / ACT copy + DVE square) — both run at 1-2x, and alternation keeps
    either engine from pacing the stream; content loads are 64-block
    grained with a fine-grained tail and the pass-1 chunk plan tapers so
    the stats tail drains quickly into the AR.
"""

import os
import sys

import numpy as np

for _p in ("/opt/trn_rl_repo",):
    if _p not in sys.path and os.path.isdir(_p):
        sys.path.insert(0, _p)

from concourse import bacc, bass, bass_utils, masks, mybir, tile

F32 = mybir.dt.float32
BF16 = mybir.dt.bfloat16
I32 = mybir.dt.int32
I16 = mybir.dt.int16

N_CORES = 8
C = 64
B = 16
ALPHA = 0.1
EPS = 1e-8

RC = 128 * 977  # per-core content rows (125056; 8*RC >= 1M)
RS = 128 * 245  # per-core style rows (31360; 8*RS >= 250K)

CH = 32    # pass-1 chunk: blocks per onehot/square tile
MMB = 2    # pass-1 chunks per matmul burst
CH2 = 32   # pass-2 chunk: blocks per out tile (4 gathers of 8)
N_PRE = 15  # pass-2 chunks whose prep is hoisted into the content-AR window


def _ema_lhsT() -> np.ndarray:
    L = np.zeros((B, B), np.float64)
    for b in range(B):
        L[b, 0] = (1.0 - ALPHA) ** b
        for j in range(1, b + 1):
            L[b, j] = ALPHA * (1.0 - ALPHA) ** (b - j)
    return np.ascontiguousarray(L.T).astype(np.float32)


def _e8() -> np.ndarray:
    # e8[g, 16g+s] = 1 : broadcast row g of a K=8 rhs to partitions 16g..16g+16
    e = np.zeros((8, 128), np.float32)
    for g in range(8):
        e[g, 16 * g : 16 * g + 16] = 1.0
    return e


def _s4() -> np.ndarray:
    # s4[32j+s, s] = 1 (s<16): sum the 4 col-group partial stats
    s = np.zeros((128, B), np.float32)
    for j in range(4):
        for t in range(B):
            s[32 * j + t, t] = 1.0
    return s


def _io16() -> np.ndarray:
    return np.broadcast_to(np.arange(B, dtype=np.int16), (128, B)).copy()


def _pid16() -> np.ndarray:
    return (np.arange(128, dtype=np.float32) % 16).reshape(128, 1)


def _chunks(total: int, step: int):
    t0 = 0
    while t0 < total:
        yield t0, min(step, total - t0)
        t0 += step


def _load_plan(ntc: int):
    # 64-block (2MB) loads with a fine-grained tail so the last pass-1
    # chunks aren't gated on one huge DMA completion.
    plan = []
    t = 0
    while ntc - t > 80:
        plan.append((t, 64))
        t += 64
    rem = ntc - t
    for sz in (32, 16, 8, 8, 4, 4, 2, 2, 1, 1):
        if rem <= 0:
            break
        take = min(sz, rem)
        plan.append((t, take))
        t += take
        rem -= take
    while rem > 0:
        plan.append((t, 1))
        t += 1
        rem -= 1
    return plan


def build_nc(rc: int = RC, rs: int = RS, n_cores: int = N_CORES):
    ntc = rc // 128
    nts = rs // 128
    ntc_pad = ((ntc + 7) // 8) * 8
    ntc_t = ((ntc + 127) // 128) * 128

    nc = bacc.Bacc(
        "TRN2", target_bir_lowering=False, debug=False, num_devices=n_cores
    )
    cf = nc.dram_tensor("cf", [rc, C], F32, kind="ExternalInput")
    ci = nc.dram_tensor("ci", [rc], I32, kind="ExternalInput")
    sf = nc.dram_tensor("sf", [rs, C], F32, kind="ExternalInput")
    si = nc.dram_tensor("si", [rs], I32, kind="ExternalInput")
    el = nc.dram_tensor("el", [B, B], F32, kind="ExternalInput")
    e8 = nc.dram_tensor("e8", [8, 128], F32, kind="ExternalInput")
    s4 = nc.dram_tensor("s4", [128, B], F32, kind="ExternalInput")
    io16 = nc.dram_tensor("io16", [128, B], I16, kind="ExternalInput")
    pid16 = nc.dram_tensor("pid16", [128, 1], F32, kind="ExternalInput")
    out = nc.dram_tensor("out", [rc, C], BF16, kind="ExternalOutput")

    cf_v = cf.ap().rearrange("(p n) d -> p n d", p=128)
    ci_v = ci.ap().rearrange("(p n) -> p n", p=128)
    sf_v = sf.ap().rearrange("(p n) d -> p n d", p=128)
    si_v = si.ap().rearrange("(p n) -> p n", p=128)
    out_v = out.ap().rearrange("(p n) d -> p n d", p=128)

    def out_q(i):
        return nc.sync if i % 2 == 0 else nc.scalar

    with tile.TileContext(nc) as tc:
        with (
            tc.tile_pool(name="const", bufs=1) as constp,
            tc.tile_pool(name="cache", bufs=1) as cachep,
            tc.tile_pool(name="dram", bufs=1, space="DRAM") as dramp,
        ):
            # identity built FIRST: its gpsimd ops must not queue behind the
            # bulk-load dma_starts (each paces with the DMA pipeline).
            ident = constp.tile([128, 128], BF16)
            masks.make_identity(nc, ident[:])
            ones_sb = constp.tile([128, 1], BF16)
            nc.gpsimd.memset(ones_sb[:], 1.0)

            # ---------- bulk loads, all on the gpsimd SWDGE queue so the
            # descriptor order controls arrival: indices first, then style,
            # then content.  Feature loads are f32->bf16 cast-DMAs into
            # resident bf16 caches. ----------
            ci_sb = constp.tile([128, ntc], I32)
            nc.gpsimd.dma_start(ci_sb[:], ci_v)
            si_sb = constp.tile([128, nts], I32)
            nc.gpsimd.dma_start(si_sb[:], si_v)
            # dummy AllReduce, triggered before the bulk stream: absorbs the
            # first-collective warmup so the real AR at the end runs ~10us.
            din = dramp.tile([B, B], F32, tag="din")
            dout = dramp.tile([B, B], F32, tag="dout")
            nc.sync.dma_start(din[:], el.ap())
            nc.gpsimd.collective_compute(
                "AllReduce",
                mybir.AluOpType.add,
                replica_groups=[list(range(n_cores))],
                ins=[din.opt()],
                outs=[dout.opt()],
            )
            sxcache = cachep.tile([128, nts, C], BF16)
            for l0, nl in _chunks(nts, 64):
                nc.gpsimd.dma_start(
                    sxcache[:, l0 : l0 + nl, :], sf_v[:, l0 : l0 + nl, :]
                )
            # content loads are split around the style-AR trigger (the
            # collective rides the gpsimd queue): loads emitted before it
            # keep the SDMA engines fed while the trigger waits.
            xcache = cachep.tile([128, ntc_pad, C], BF16)
            load_list = _load_plan(ntc)

            def emit_content_loads(lo, hi):
                for l0, nl in load_list[lo:hi]:
                    nc.gpsimd.dma_start(
                        xcache[:, l0 : l0 + nl, :], cf_v[:, l0 : l0 + nl, :]
                    )

            emit_content_loads(0, None)

            # ---------- constants ----------
            el_sb = constp.tile([B, B], F32)
            nc.sync.dma_start(el_sb[:], el.ap())
            e8_sbf = constp.tile([8, 128], F32)
            nc.sync.dma_start(e8_sbf[:], e8.ap())
            e8_sb = constp.tile([8, 128], BF16)
            nc.vector.tensor_copy(e8_sb[:], e8_sbf[:])
            s4_sb = constp.tile([128, B], F32)
            nc.sync.dma_start(s4_sb[:], s4.ap())
            io16_sb = constp.tile([128, B], I16)
            nc.sync.dma_start(io16_sb[:], io16.ap())
            pid_sb = constp.tile([128, 1], F32)
            nc.sync.dma_start(pid_sb[:], pid16.ap())

            if ntc_pad > ntc:
                nc.vector.memset(xcache[:, ntc:ntc_pad, :], 0.0)

            # coef_bd zero-fill happens early (off the post-AR critical path)
            coef_bd_a = constp.tile([128, 8 * C], BF16)
            coef_bd_b = constp.tile([128, 8 * C], BF16)
            nc.vector.memset(coef_bd_a[:], 0.0)
            nc.vector.memset(coef_bd_b[:], 0.0)

            # ---------- transposed indices -> ciT8 in DRAM, k-major layout:
            # ciT8[g, (k*Q+q)*128 + p] = ciT[8k+g, 128q+p] = idx of lane p in
            # block 128q+8k+g.  k-major makes each regroup DMA a contiguous
            # 2KB-per-partition write (the q-major layout wrote 256B runs,
            # which crawl on HBM). ----------
            NQ = ntc_t // 128
            idxbf = constp.tile([128, ntc_t], BF16)
            if ntc_t > ntc:
                nc.vector.memset(idxbf[:, ntc:ntc_t], float(B))
            nc.vector.tensor_copy(idxbf[:, 0:ntc], ci_sb[:])
            ciT = constp.tile([128, ntc_t], BF16)
            ciT8_d = dramp.tile([8, 16 * NQ * 128], BF16, tag="ciT8")
            with tc.tile_pool(name="ps_tr", bufs=2, space="PSUM") as pstr:
                for q in range(NQ):
                    psT = pstr.tile([128, 128], BF16, tag="tr")
                    nc.tensor.transpose(
                        psT[:], idxbf[:, 128 * q : 128 * (q + 1)], ident[:]
                    )
                    nc.scalar.copy(ciT[:, 128 * q : 128 * (q + 1)], psT[:])
            ciT8_kv = ciT8_d[:].rearrange("g (k r) -> g k r", r=NQ * 128)
            for k in range(16):
                nc.sync.dma_start(
                    ciT8_kv[:, k, :], ciT[8 * k : 8 * k + 8, :]
                )
            # read view for pass-2 prep: [g, k, q, p]
            ciT8_rv = ciT8_d[:].rearrange("g (k q p) -> g k q p", q=NQ, p=128)

            # ---------- pass-1 helper ----------
            def p1_chunk_plan(nt_total):
                # 32-block chunks with a fine tail so the last stats chunks
                # drain through the pipeline quickly.
                plan = []
                t = 0
                while nt_total - t > 48:
                    plan.append((t, 32))
                    t += 32
                rem = nt_total - t
                for sz in (16, 8, 8, 4, 4, 2, 2, 1, 1, 1, 1, 1, 1, 1, 1):
                    if rem <= 0:
                        break
                    take = min(sz, rem)
                    plan.append((t, take))
                    t += take
                    rem -= take
                return plan

            def pass1(x_chunk, idx_sb, nt_total, ps, p1w, p1o, tail, off=0):
                """Segment sums into ps[32j+s] = [sum x | sum x^2 | count] for
                blocks t%4==j.  One 129-col matmul per block; rhs built by
                copy+square alternating between DVE and ACT per chunk."""
                tot = [0, 0, 0, 0]
                for t in range(nt_total):
                    tot[t % 4] += 1
                n_mm = [0, 0, 0, 0]
                pending = []

                def flush():
                    for t0, nb, rhs, oh in pending:
                        for k in range(nb):
                            j = (t0 + k) % 4
                            nc.tensor.matmul(
                                ps[32 * j : 32 * j + B, 0 : 2 * C + 1],
                                oh[:, k, :],
                                rhs[:, k, 0 : 2 * C + 1],
                                start=(n_mm[j] == 0),
                                stop=(n_mm[j] == tot[j] - 1),
                                tile_position=(0, 32 * j),
                                skip_group_check=True,
                            )
                            n_mm[j] += 1
                    pending.clear()

                plan = p1_chunk_plan(nt_total) if tail else list(
                    _chunks(nt_total, CH)
                )
                for ck, (t0, nb) in enumerate(plan):
                    x_ap = x_chunk(ck, t0, nb)
                    rhs = p1w.tile([128, CH, 132], BF16, tag="p1r")
                    if ck < MMB + 1:
                        nc.vector.memset(rhs[:, :, 2 * C : 2 * C + 1], 1.0)
                    if ck % 2 == 0:
                        nc.vector.tensor_copy(rhs[:, :nb, 0:C], x_ap)
                        nc.scalar.activation(
                            rhs[:, :nb, C : 2 * C],
                            x_ap,
                            mybir.ActivationFunctionType.Square,
                        )
                    else:
                        nc.scalar.activation(
                            rhs[:, :nb, 0:C],
                            x_ap,
                            mybir.ActivationFunctionType.Copy,
                        )
                        nc.vector.tensor_tensor(
                            rhs[:, :nb, C : 2 * C], x_ap, x_ap,
                            mybir.AluOpType.mult,
                        )
                    oh = p1o.tile([128, CH, B], BF16, tag="p1o")
                    nc.vector.tensor_tensor(
                        oh[:, :nb, :],
                        idx_sb[:, off + t0 : off + t0 + nb]
                        .unsqueeze(2)
                        .broadcast_to((128, nb, B)),
                        io16_sb[:].unsqueeze(1).broadcast_to((128, nb, B)),
                        mybir.AluOpType.is_equal,
                    )
                    pending.append((t0, nb, rhs, oh))
                    if len(pending) >= MMB:
                        flush()
                flush()

            def merge_stats(ps, psel, dst_sb):
                ev = constp.tile([128, 2 * C + 1], F32, tag="ev")
                nc.vector.memset(ev[:], 0.0)
                for j in range(4):
                    nc.scalar.copy(
                        ev[32 * j : 32 * j + B, :],
                        ps[32 * j : 32 * j + B, 0 : 2 * C + 1],
                    )
                nc.tensor.matmul(
                    psel[0:B, 0 : 2 * C + 1], s4_sb[:], ev[:], start=True,
                    stop=True,
                )
                nc.scalar.copy(dst_sb, psel[0:B, 0 : 2 * C + 1])

            def ar_start(src_sb, tag, w):
                inb = dramp.tile([B, w], F32, tag=f"in_{tag}")
                outb = dramp.tile([B, w], F32, tag=f"out_{tag}")
                nc.sync.dma_start(inb[:], src_sb)
                nc.gpsimd.collective_compute(
                    "AllReduce",
                    mybir.AluOpType.add,
                    replica_groups=[list(range(n_cores))],
                    ins=[inb.opt()],
                    outs=[outb.opt()],
                )
                return outb

            def seg_stats(g, mean_out, std_out):
                sums, ssq, cnt = g[:, 0:C], g[:, C : 2 * C], g[:, 2 * C : 2 * C + 1]
                rc_ = constp.tile([B, 1], F32, tag="t1")
                nc.vector.reciprocal(rc_[:], cnt)
                nm1 = constp.tile([B, 1], F32, tag="t2")
                nc.vector.tensor_scalar_add(nm1[:], cnt, -1.0)
                rnm1 = constp.tile([B, 1], F32, tag="t3")
                nc.vector.reciprocal(rnm1[:], nm1[:])
                fac = constp.tile([B, 1], F32, tag="t4")
                nc.vector.tensor_tensor(fac[:], cnt, rnm1[:], mybir.AluOpType.mult)
                nc.vector.tensor_scalar_mul(mean_out, sums, rc_[:])
                ex2 = constp.tile([B, C], F32, tag="t5")
                nc.vector.tensor_scalar_mul(ex2[:], ssq, rc_[:])
                m2 = constp.tile([B, C], F32, tag="t6")
                nc.scalar.square(m2[:], mean_out)
                var = constp.tile([B, C], F32, tag="t7")
                nc.vector.tensor_sub(var[:], ex2[:], m2[:])
                nc.vector.tensor_scalar_mul(var[:], var[:], fac[:])
                nc.vector.tensor_scalar_max(var[:], var[:], 0.0)
                nc.scalar.sqrt(std_out, var[:])
                nc.vector.tensor_scalar_add(std_out, std_out, EPS)

            gm_t = constp.tile([B, C], F32)
            gs_t = constp.tile([B, C], F32)
            stat2 = constp.tile([B, 2 * (2 * C + 1)], F32)

            with (
                tc.tile_pool(name="p1w", bufs=MMB + 1) as p1w,
                tc.tile_pool(name="p1o", bufs=MMB + 1) as p1o,
                tc.tile_pool(name="ps_p1", bufs=1, space="PSUM") as psp,
                tc.tile_pool(name="ps_sel", bufs=1, space="PSUM") as psel_p,
            ):
                # ---------- style pass 1 + early AR (hidden under content
                # stream-in) ----------
                ps_s = psp.tile([128, 512], F32, tag="ps_s")
                pass1(
                    lambda ck, t0, nb: sxcache[:, t0 : t0 + nb, :],
                    si_sb, nts, ps_s, p1w, p1o, False,
                )
                psel = psel_p.tile([128, 2 * C + 1], F32, tag="psel")
                merge_stats(ps_s, psel, stat2[:, 0 : 2 * C + 1])

                # ---------- content pass 1 ----------
                ps_c = psp.tile([128, 512], F32, tag="ps_c")
                pass1(
                    lambda ck, t0, nb: xcache[:, t0 : t0 + nb, :],
                    ci_sb, ntc, ps_c, p1w, p1o, True,
                )
                psel2 = psel_p.tile([128, 2 * C + 1], F32, tag="psel")
                merge_stats(ps_c, psel2, stat2[:, 2 * C + 1 :])
                outb_c = ar_start(stat2[:], "sc", w=2 * (2 * C + 1))

            # ---------- pass 2 ----------
            chunk_list = list(_chunks(ntc_pad, CH2))

            with (
                tc.tile_pool(name="p2ct", bufs=3) as p2ct,
                tc.tile_pool(name="p2oh", bufs=N_PRE + 2) as p2oh,
                tc.tile_pool(name="p2xb", bufs=2) as p2xb,
                tc.tile_pool(name="p2out", bufs=2) as p2out,
                tc.tile_pool(name="ps_b", bufs=2, space="PSUM") as psb_p,
                tc.tile_pool(name="ps_ga", bufs=2, space="PSUM") as psga_p,
                tc.tile_pool(name="ps_gb", bufs=2, space="PSUM") as psgb_p,
            ):
                def p2_prep(t0, nb):
                    """index slice load + K=8 broadcast MM + per-partition
                    compare -> transposed one-hot strips for nb blocks."""
                    ngr = nb // 8
                    w = ngr * 128
                    g0 = t0 // 8
                    q0, k0 = g0 // 16, g0 % 16
                    ct8 = p2ct.tile([8, (CH2 // 8) * 128], BF16, tag="ct8")
                    nc.sync.dma_start(
                        ct8[:, 0:w].rearrange("g (k p) -> g k p", p=128),
                        ciT8_rv[:, k0 : k0 + ngr, q0, :],
                    )
                    psB = psb_p.tile([128, 512], F32, tag="bc")
                    nc.tensor.matmul(
                        psB[:, 0:w], e8_sb[:], ct8[:, 0:w], start=True, stop=True
                    )
                    ohT = p2oh.tile([128, 512], BF16, tag="ohT")
                    nc.vector.tensor_scalar(
                        ohT[:, 0:w],
                        psB[:, 0:w],
                        pid_sb[:],
                        None,
                        mybir.AluOpType.is_equal,
                    )
                    return ohT

                # prep for the first chunks runs during the content AR
                preps = {}
                for ck in range(min(N_PRE, len(chunk_list))):
                    t0, nb = chunk_list[ck]
                    preps[ck] = p2_prep(t0, nb)

                # head-AR out -> style EMA math runs during the tail AR;
                # tail-AR out is then folded into the content stats.
                gstat2 = constp.tile([B, 2 * (2 * C + 1)], F32)
                nc.sync.dma_start(gstat2[:], outb_c[:])
                s_stats = constp.tile([B, 2 * C], F32)
                seg_stats(
                    gstat2[:, 0 : 2 * C + 1], s_stats[:, 0:C],
                    s_stats[:, C : 2 * C],
                )
                g_ps = psga_p.tile([128, 1024], F32, tag="ga")
                nc.tensor.matmul(
                    g_ps[0:B, 0 : 2 * C], el_sb[:], s_stats[:], start=True,
                    stop=True,
                )
                nc.vector.tensor_copy(gm_t[:], g_ps[0:B, 0:C])
                nc.vector.tensor_copy(gs_t[:], g_ps[0:B, C : 2 * C])
                mean_c = constp.tile([B, C], F32)
                std_c = constp.tile([B, C], F32)
                seg_stats(gstat2[:, 2 * C + 1 :], mean_c[:], std_c[:])
                rstd = constp.tile([B, C], F32)
                nc.vector.reciprocal(rstd[:], std_c[:])
                a_t = constp.tile([B, C], F32)
                nc.vector.tensor_tensor(
                    a_t[:], gs_t[:], rstd[:], mybir.AluOpType.mult
                )
                tmp = constp.tile([B, C], F32)
                nc.vector.tensor_tensor(
                    tmp[:], mean_c[:], a_t[:], mybir.AluOpType.mult
                )
                b_t = constp.tile([B, C], F32)
                nc.vector.tensor_sub(b_t[:], gm_t[:], tmp[:])
                ra_t = constp.tile([B, C], F32)
                nc.vector.reciprocal(ra_t[:], a_t[:])
                bp_t = constp.tile([B, C], F32)
                nc.vector.tensor_tensor(
                    bp_t[:], b_t[:], ra_t[:], mybir.AluOpType.mult
                )
                coef_a = constp.tile([B, C], BF16)
                nc.vector.tensor_copy(coef_a[:], a_t[:])
                coef_bp = constp.tile([B, C], BF16)
                nc.vector.tensor_copy(coef_bp[:], bp_t[:])
                # block-diagonal spread: row 16g+s holds coef[s] at cols
                # [64g, 64g+64); zero elsewhere (zero-filled early above).
                _q3 = [nc.sync, nc.scalar, nc.gpsimd]
                for g in range(8):
                    _q3[(2 * g) % 3].dma_start(
                        coef_bd_a[16 * g : 16 * g + B, C * g : C * g + C],
                        coef_a[:],
                    )
                    _q3[(2 * g + 1) % 3].dma_start(
                        coef_bd_b[16 * g : 16 * g + B, C * g : C * g + C],
                        coef_bp[:],
                    )

                # ---------- pass-2 main loop ----------
                # even groups: PE identity-MM accumulates x onto the b'
                # gather (PSUM), ACT evacs (x+b').  odd groups: b'-gather
                # only, ACT evacs b', GPSIMD adds x (spreads the +x between
                # the otherwise-idle GPSIMD and the PE).
                n_ga = 0
                for ck, (t0, nb) in enumerate(chunk_list):
                    ngr = nb // 8
                    ohT = preps[ck] if ck in preps else p2_prep(t0, nb)
                    ot = p2out.tile([128, CH2, C], BF16, tag="p2o")
                    for pair0 in range(0, ngr, 2):
                        pu = list(range(pair0, min(pair0 + 2, ngr)))
                        np_ = len(pu)
                        # psA holds both groups' a-gathers (one bank each);
                        # the pair shares one DVE mult and one xb pair tile.
                        psA = psga_p.tile([128, 1024], F32, tag="ga")
                        psBs = {}
                        kinds = {}
                        for i_u, u in enumerate(pu):
                            n_ga += 1
                            kinds[u] = n_ga % 2
                            psB2 = psgb_p.tile([128, 512], F32, tag="gb")
                            nc.tensor.matmul(
                                psA[:, 512 * i_u : 512 * (i_u + 1)],
                                ohT[:, u * 128 : (u + 1) * 128],
                                coef_bd_a[:],
                                start=True,
                                stop=True,
                                skip_group_check=True,
                            )
                            nc.tensor.matmul(
                                psB2[:],
                                ohT[:, u * 128 : (u + 1) * 128],
                                coef_bd_b[:],
                                start=True,
                                stop=(kinds[u] == 1),
                                skip_group_check=True,
                            )
                            psBs[u] = psB2
                        # identity x-accumulates for even groups only
                        for u in pu:
                            if kinds[u] == 1:
                                continue
                            b0 = t0 + 8 * u
                            nc.tensor.matmul(
                                psBs[u][:],
                                ident[:],
                                xcache[:, b0 : b0 + 8, :].rearrange(
                                    "p n d -> p (n d)"
                                ),
                                start=False,
                                stop=True,
                                skip_group_check=True,
                            )
                        xbp = p2xb.tile([128, 16, C], BF16, tag="xb")
                        for i_u, u in enumerate(pu):
                            b0 = t0 + 8 * u
                            if kinds[u] == 1:
                                xb0 = p2xb.tile([128, 8, C], BF16, tag="xb0")
                                nc.scalar.copy(
                                    xb0[:],
                                    psBs[u][:].rearrange(
                                        "p (n d) -> p n d", d=C
                                    ),
                                )
                                nc.gpsimd.tensor_tensor(
                                    xbp[:, 8 * i_u : 8 * i_u + 8, :],
                                    xb0[:],
                                    xcache[:, b0 : b0 + 8, :],
                                    mybir.AluOpType.add,
                                )
                            else:
                                nc.scalar.copy(
                                    xbp[:, 8 * i_u : 8 * i_u + 8, :],
                                    psBs[u][:].rearrange(
                                        "p (n d) -> p n d", d=C
                                    ),
                                )
                        nc.vector.tensor_tensor(
                            ot[:, 8 * pair0 : 8 * (pair0 + np_), :],
                            psA[:, 0 : 512 * np_].rearrange(
                                "p (n d) -> p n d", d=C
                            ),
                            xbp[:, 0 : 8 * np_, :],
                            mybir.AluOpType.mult,
                        )
                    nreal = min(nb, ntc - t0)
                    if nreal > 0:
                        out_q(ck).dma_start(
                            out_v[:, t0 : t0 + nreal, :], ot[:, :nreal, :]
                        )

    nc.compile()
    return nc


_NC_CACHE = {}


def _get_nc(rc=RC, rs=RS, n_cores=N_CORES):
    key = (rc, rs, n_cores)
    if key not in _NC_CACHE:
        _NC_CACHE[key] = build_nc(rc, rs, n_cores)
    return _NC_CACHE[key]


def _pad_rows(a: np.ndarray, total: int, fill) -> np.ndarray:
    pad = total - a.shape[0]
    if pad == 0:
        return np.ascontiguousarray(a)
    pad_shape = (pad,) + a.shape[1:]
    return np.concatenate([a, np.full(pad_shape, fill, a.dtype)], axis=0)


def make_in_maps(cf, ci, sf, si, rc=RC, rs=RS, n_cores=N_CORES):
    cf = _pad_rows(np.asarray(cf, np.float32), n_cores * rc, 0.0)
    ci = _pad_rows(np.asarray(ci, np.int32), n_cores * rc, B)
    sf = _pad_rows(np.asarray(sf, np.float32), n_cores * rs, 0.0)
    si = _pad_rows(np.asarray(si, np.int32), n_cores * rs, B)
    el = _ema_lhsT()
    e8 = _e8()
    s4 = _s4()
    io16 = _io16()
    pid16 = _pid16()
    return [
        {
            "cf": np.ascontiguousarray(cf[k * rc : (k + 1) * rc]),
            "ci": np.ascontiguousarray(ci[k * rc : (k + 1) * rc]),
            "sf": np.ascontiguousarray(sf[k * rs : (k + 1) * rs]),
            "si": np.ascontiguousarray(si[k * rs : (k + 1) * rs]),
            "el": el,
            "e8": e8,
            "s4": s4,
            "io16": io16,
            "pid16": pid16,
        }
        for k in range(n_cores)
    ]


def kernel(
    content_feats: np.ndarray,
    style_feats: np.ndarray,
    content_batch_indices: np.ndarray,
    style_batch_indices: np.ndarray,
    num_batches=B,
) -> np.ndarray:
    n_c = content_feats.shape[0]
    nc = _get_nc()
    in_maps = make_in_maps(
        content_feats, content_batch_indices, style_feats, style_batch_indices
    )
    res = bass_utils.run_bass_kernel_spmd(nc, in_maps, core_ids=list(range(N_CORES)))
    out = np.concatenate(
        [np.asarray(res.results[k]["out"]) for k in range(N_CORES)], axis=0
    )
    return np.ascontiguousarray(out[:n_c]).astype(np.float32)
